# revision 32
# baseline (speedup 1.0000x reference)
"""AttractorDynamics Trainium2 kernel.

Reference computation (B=16384, M=1024, A=512, tau=0.1, 10 settling steps):
    drive = x @ W_in.T + b_in                     # [B, A]
    sigma = 0
    repeat 10: sigma = tanh(drive + (sigma @ J.T) / tau)

Strategy: data-parallel over batch across 8 NeuronCores (2048 rows each).
Everything is kept in a transposed [feature, batch] layout so every matmul
has the contraction dim on partitions and no on-device transposes are
needed:
    drive^T[a, b] = sum_m W_in^T[m, a] * x^T[m, b]
    (sigma @ Js^T)^T[a', b] = sum_a Js^T[a, a'] * sigma^T[a, b]
with Js = J / tau folded on the host. Matmul operands are bf16 (full-rate
PE, fast weight loads) with fp32 PSUM accumulation; drive is kept fp32 and
the final settling step is computed and stored in fp32.

All DRAM tensors are pre-arranged on the host into SBUF-image layout
([128 partitions, flat free dim], per-tile contiguous) so every DMA moves
128 rows of multi-KB contiguous data at full queue bandwidth.

Per core the 2048 batch columns are cut into tiles [256, 512, 512, 512,
256] processed as two interleaved groups (3 then 2 tiles) so the tensor
engine always has another tile's matmuls to run while a tile's add+tanh
tail executes. The narrow first tile gets the first matmul started sooner
after the DMA preamble; the narrow last tile shortens the final drain.
Output chunks stream out on alternating DMA queues as soon as their final
tanh lands.
"""

import numpy as np

B, M, A = 16384, 1024, 512
TAU = 0.1
STEPS = 10
NCORES = 8
BSH = B // NCORES  # 2048 batch rows per core
KM = M // 128  # 8 contraction chunks for the input projection
KA = A // 128  # 4 contraction chunks for the recurrence

# (column offset, width) per batch tile; groups are processed sequentially
# with round-robin settling inside each group.
TILES = [(0, 256), (256, 512), (768, 512), (1280, 512), (1792, 256)]
GROUPS = [(0, 1, 2), (3, 4)]
# start of each tile's block in the tile-contiguous x layout (in elements
# of the free dim, per partition)
XPOS = [off * KM for off, _ in TILES]

_CACHE = {}


def _build_nc():
    import sys

    for p in ("/opt/trn_rl_repo",):
        if p not in sys.path:
            sys.path.append(p)
    import concourse.tile as tile
    from concourse import bacc, mybir

    f32 = mybir.dt.float32
    bf16 = mybir.dt.bfloat16
    Tanh = mybir.ActivationFunctionType.Tanh

    nc = bacc.Bacc(None)
    # All tensors in [128, flat] SBUF-image layout (see _make_in_maps).
    xd = nc.dram_tensor("xd", [128, KM * BSH], bf16, kind="ExternalInput")
    wd = nc.dram_tensor("wd", [128, KM * A], bf16, kind="ExternalInput")
    jd = nc.dram_tensor("jd", [128, KA * A], bf16, kind="ExternalInput")
    bd = nc.dram_tensor("bd", [128, KA], f32, kind="ExternalInput")
    sd = nc.dram_tensor("sd", [128, KA * BSH], f32, kind="ExternalOutput")

    with (
        tile.TileContext(nc) as tc,
        tc.tile_pool(name="consts", bufs=1) as consts,
        tc.tile_pool(name="xp", bufs=5) as xp,
        tc.tile_pool(name="dp", bufs=5) as dp,
        tc.tile_pool(name="sp", bufs=8) as sp,
        tc.tile_pool(name="fp", bufs=3) as fp,
        tc.tile_pool(name="psp", bufs=8, space="PSUM") as psp,
    ):
        # w is stored ma-major: block (ma, k) of [128, 128] lives at
        # [:, (ma*KM + k)*128]; the first drive group only needs the ma=0
        # quarter, so quarter-wise loads unblock the first matmuls early.
        w_t = consts.tile([128, KA * KM * 128], bf16)
        j_t = consts.tile([128, KA * A], bf16)
        b_t = consts.tile([128, KA], f32)

        def load_x(ti, nsplit, engines=None):
            _, w = TILES[ti]
            x_t = xp.tile([128, KM * w], bf16, tag="x_t")
            step = KM * w // nsplit
            for h in range(nsplit):
                eng = engines[h] if engines else nc.sync
                eng.dma_start(
                    out=x_t[:, h * step : (h + 1) * step],
                    in_=xd[:, XPOS[ti] + h * step : XPOS[ti] + (h + 1) * step],
                )
            return x_t

        def drive_and_sig1(ti, x_t):
            # drive = x @ W_in.T + b (kept for all steps); sigma_1 = tanh(drive)
            _, w = TILES[ti]
            d_t = dp.tile([128, KA, w], f32, tag="d_t")
            s_t = sp.tile([128, KA, w], bf16, tag="s_t")
            for ma in range(KA):
                ps = psp.tile([128, w], f32, tag="ps")
                for k in range(KM):
                    blk = (ma * KM + k) * 128
                    nc.tensor.matmul(
                        ps,
                        lhsT=w_t[:, blk : blk + 128],
                        rhs=x_t[:, k * w : (k + 1) * w],
                        start=(k == 0),
                        stop=(k == KM - 1),
                    )
                nc.vector.tensor_copy(d_t[:, ma, :], ps)
                nc.scalar.activation(
                    out=s_t[:, ma, :],
                    in_=ps,
                    func=Tanh,
                    bias=b_t[:, ma : ma + 1],
                    scale=1.0,
                )
            return d_t, s_t

        def settle_step(ti, d_t, s_t, last):
            off, w = TILES[ti]
            if last:
                s_new = fp.tile([128, KA, w], f32, tag="f_t")
            else:
                s_new = sp.tile([128, KA, w], bf16, tag="s_t")
            for ma in range(KA):
                ps = psp.tile([128, w], f32, tag="ps")
                for ka in range(KA):
                    nc.tensor.matmul(
                        ps,
                        lhsT=j_t[:, ka * A + ma * 128 : ka * A + (ma + 1) * 128],
                        rhs=s_t[:, ka, :],
                        start=(ka == 0),
                        stop=(ka == KA - 1),
                    )
                nc.vector.tensor_add(s_new[:, ma, :], ps, d_t[:, ma, :])
                nc.scalar.activation(
                    out=s_new[:, ma, :],
                    in_=s_new[:, ma, :],
                    func=Tanh,
                    bias=b_t[:, ma : ma + 1],
                    scale=1.0,
                )
                if last:
                    # Stream each finished chunk out immediately via the
                    # hardware-DGE path so the final flush is fast.
                    nc.sync.dma_start(
                        out=sd[:, ma * BSH + off : ma * BSH + off + w],
                        in_=s_new[:, ma, :],
                    )
            return s_new

        # DMA issue order follows consumption order: the ma=0 W quarter and
        # the narrow x0 tile gate the first matmul group; later quarters and
        # tiles arrive while earlier groups run; j/b are needed only once
        # settling starts.
        QW = KM * 128  # columns per w quarter (one ma)

        def load_w(lo, hi):
            nc.sync.dma_start(out=w_t[:, lo:hi], in_=wd[:, lo:hi])

        load_w(0, QW)
        xs = {0: load_x(0, 2)}
        load_w(QW, 2 * QW)
        load_w(2 * QW, 3 * QW)
        load_w(3 * QW, 4 * QW)
        xs[1] = load_x(1, 2)
        nc.sync.dma_start(out=j_t, in_=jd.ap())
        xs[2] = load_x(2, 2)
        nc.sync.dma_start(out=b_t, in_=bd.ap())

        for gi, group in enumerate(GROUPS):
            dts, sts = {}, {}
            for ti in group:
                dts[ti], sts[ti] = drive_and_sig1(ti, xs.pop(ti))
            if gi + 1 < len(GROUPS):
                for ti in GROUPS[gi + 1]:
                    xs[ti] = load_x(ti, 2)
            for step in range(STEPS - 1):
                for ti in group:
                    sts[ti] = settle_step(
                        ti, dts[ti], sts[ti], last=step == STEPS - 2
                    )

    nc.finalize()
    return nc


def _get_nc():
    if "nc" not in _CACHE:
        _CACHE["nc"] = _build_nc()
    return _CACHE["nc"]


def _run(in_maps, **kwargs):
    import sys

    for p in ("/opt/trn_rl_repo",):
        if p not in sys.path:
            sys.path.append(p)
    from concourse.bass_utils import run_bass_kernel_spmd

    return run_bass_kernel_spmd(_get_nc(), in_maps, list(range(NCORES)), **kwargs)


def _to_sbuf_image(mat, kchunks):
    """[kchunks*128, F] -> [128, kchunks*F] (partition-major SBUF image)."""
    k, f = kchunks, mat.shape[1]
    return (
        mat.reshape(k, 128, f).transpose(1, 0, 2).reshape(128, k * f)
    )


def _make_in_maps(x, W_in, b_in, J):
    import ml_dtypes

    bf = ml_dtypes.bfloat16
    x = np.asarray(x, dtype=np.float32)
    # w image is ma-major: [128, (ma, k, col)] with block (ma, k) holding
    # W_in^T[k*128 + p, ma*128 + col].
    wT = np.asarray(W_in, dtype=np.float32).T.reshape(KM, 128, KA, 128)
    wd = np.ascontiguousarray(
        wT.transpose(1, 2, 0, 3).reshape(128, KA * KM * 128)
    ).astype(bf)
    jd = np.ascontiguousarray(
        _to_sbuf_image(np.asarray(J, dtype=np.float32).T / TAU, KA)
    ).astype(bf)
    bd = np.ascontiguousarray(
        np.asarray(b_in, dtype=np.float32).reshape(KA, 128).T
    )
    in_maps = []
    for c in range(NCORES):
        xT = x[c * BSH : (c + 1) * BSH, :].T  # [M, BSH]
        xpk = xT.reshape(KM, 128, BSH).transpose(1, 0, 2)  # [128, KM, BSH]
        blocks = [
            xpk[:, :, off : off + w].reshape(128, KM * w) for off, w in TILES
        ]
        xdc = np.ascontiguousarray(np.concatenate(blocks, axis=1)).astype(bf)
        in_maps.append({"xd": xdc, "wd": wd, "jd": jd, "bd": bd})
    return in_maps


def _assemble(results):
    # sd [128, KA*BSH] per core -> sigma^T [A, BSH] -> concat -> [B, A]
    parts = []
    for c in range(NCORES):
        sdc = results[c]["sd"].reshape(128, KA, BSH)
        parts.append(sdc.transpose(1, 0, 2).reshape(A, BSH))
    sigT = np.concatenate(parts, axis=1)
    return np.ascontiguousarray(sigT.T).astype(np.float32)  # [B, A]


def kernel(x, W_in, b_in, J):
    res = _run(_make_in_maps(x, W_in, b_in, J))
    return _assemble(res.results)


# revision 33
# speedup vs baseline: 1.0032x; 1.0032x over previous
"""AttractorDynamics Trainium2 kernel.

Reference computation (B=16384, M=1024, A=512, tau=0.1, 10 settling steps):
    drive = x @ W_in.T + b_in                     # [B, A]
    sigma = 0
    repeat 10: sigma = tanh(drive + (sigma @ J.T) / tau)

Strategy: data-parallel over batch across 8 NeuronCores (2048 rows each).
Everything is kept in a transposed [feature, batch] layout so every matmul
has the contraction dim on partitions and no on-device transposes are
needed:
    drive^T[a, b] = sum_m W_in^T[m, a] * x^T[m, b]
    (sigma @ Js^T)^T[a', b] = sum_a Js^T[a, a'] * sigma^T[a, b]
with Js = J / tau folded on the host. Matmul operands are bf16 (full-rate
PE, fast weight loads) with fp32 PSUM accumulation; drive is kept fp32 and
the final settling step is computed and stored in fp32.

All DRAM tensors are pre-arranged on the host into SBUF-image layout
([128 partitions, flat free dim], per-tile contiguous) so every DMA moves
128 rows of multi-KB contiguous data at full queue bandwidth.

Per core the 2048 batch columns are cut into tiles [256, 512, 512, 512,
256] processed as two interleaved groups (3 then 2 tiles) so the tensor
engine always has another tile's matmuls to run while a tile's add+tanh
tail executes. The narrow first tile gets the first matmul started sooner
after the DMA preamble; the narrow last tile shortens the final drain.
Output chunks stream out on alternating DMA queues as soon as their final
tanh lands.
"""

import numpy as np

B, M, A = 16384, 1024, 512
TAU = 0.1
STEPS = 10
NCORES = 8
BSH = B // NCORES  # 2048 batch rows per core
KM = M // 128  # 8 contraction chunks for the input projection
KA = A // 128  # 4 contraction chunks for the recurrence

# (column offset, width) per batch tile; groups are processed sequentially
# with round-robin settling inside each group.
TILES = [(0, 512), (512, 512), (1024, 512), (1536, 512)]
GROUPS = [(0, 1), (2, 3)]
# start of each tile's block in the tile-contiguous x layout (in elements
# of the free dim, per partition)
XPOS = [off * KM for off, _ in TILES]

_CACHE = {}


def _build_nc():
    import sys

    for p in ("/opt/trn_rl_repo",):
        if p not in sys.path:
            sys.path.append(p)
    import concourse.tile as tile
    from concourse import bacc, mybir

    f32 = mybir.dt.float32
    bf16 = mybir.dt.bfloat16
    Tanh = mybir.ActivationFunctionType.Tanh

    nc = bacc.Bacc(None)
    # All tensors in [128, flat] SBUF-image layout (see _make_in_maps).
    xd = nc.dram_tensor("xd", [128, KM * BSH], bf16, kind="ExternalInput")
    wd = nc.dram_tensor("wd", [128, KM * A], bf16, kind="ExternalInput")
    jd = nc.dram_tensor("jd", [128, KA * A], bf16, kind="ExternalInput")
    bd = nc.dram_tensor("bd", [128, KA], f32, kind="ExternalInput")
    sd = nc.dram_tensor("sd", [128, KA * BSH], f32, kind="ExternalOutput")

    with (
        tile.TileContext(nc) as tc,
        tc.tile_pool(name="consts", bufs=1) as consts,
        tc.tile_pool(name="xp", bufs=5) as xp,
        tc.tile_pool(name="dp", bufs=5) as dp,
        tc.tile_pool(name="sp", bufs=8) as sp,
        tc.tile_pool(name="fp", bufs=3) as fp,
        tc.tile_pool(name="psp", bufs=8, space="PSUM") as psp,
    ):
        # w is stored ma-major: block (ma, k) of [128, 128] lives at
        # [:, (ma*KM + k)*128]; the first drive group only needs the ma=0
        # quarter, so quarter-wise loads unblock the first matmuls early.
        w_t = consts.tile([128, KA * KM * 128], bf16)
        j_t = consts.tile([128, KA * A], bf16)
        b_t = consts.tile([128, KA], f32)

        def load_x(ti, nsplit, engines=None):
            _, w = TILES[ti]
            x_t = xp.tile([128, KM * w], bf16, tag="x_t")
            step = KM * w // nsplit
            for h in range(nsplit):
                eng = engines[h] if engines else nc.sync
                eng.dma_start(
                    out=x_t[:, h * step : (h + 1) * step],
                    in_=xd[:, XPOS[ti] + h * step : XPOS[ti] + (h + 1) * step],
                )
            return x_t

        def drive_and_sig1(ti, x_t):
            # drive = x @ W_in.T + b (kept for all steps); sigma_1 = tanh(drive)
            _, w = TILES[ti]
            d_t = dp.tile([128, KA, w], f32, tag="d_t")
            s_t = sp.tile([128, KA, w], bf16, tag="s_t")
            for ma in range(KA):
                ps = psp.tile([128, w], f32, tag="ps")
                for k in range(KM):
                    blk = (ma * KM + k) * 128
                    nc.tensor.matmul(
                        ps,
                        lhsT=w_t[:, blk : blk + 128],
                        rhs=x_t[:, k * w : (k + 1) * w],
                        start=(k == 0),
                        stop=(k == KM - 1),
                    )
                nc.vector.tensor_copy(d_t[:, ma, :], ps)
                nc.scalar.activation(
                    out=s_t[:, ma, :],
                    in_=ps,
                    func=Tanh,
                    bias=b_t[:, ma : ma + 1],
                    scale=1.0,
                )
            return d_t, s_t

        def settle_step(ti, d_t, s_t, last):
            off, w = TILES[ti]
            if last:
                s_new = fp.tile([128, KA, w], f32, tag="f_t")
            else:
                s_new = sp.tile([128, KA, w], bf16, tag="s_t")
            for ma in range(KA):
                ps = psp.tile([128, w], f32, tag="ps")
                for ka in range(KA):
                    nc.tensor.matmul(
                        ps,
                        lhsT=j_t[:, ka * A + ma * 128 : ka * A + (ma + 1) * 128],
                        rhs=s_t[:, ka, :],
                        start=(ka == 0),
                        stop=(ka == KA - 1),
                    )
                nc.vector.tensor_add(s_new[:, ma, :], ps, d_t[:, ma, :])
                nc.scalar.activation(
                    out=s_new[:, ma, :],
                    in_=s_new[:, ma, :],
                    func=Tanh,
                    bias=b_t[:, ma : ma + 1],
                    scale=1.0,
                )
                if last:
                    # Stream each finished chunk out immediately via the
                    # hardware-DGE path so the final flush is fast.
                    nc.sync.dma_start(
                        out=sd[:, ma * BSH + off : ma * BSH + off + w],
                        in_=s_new[:, ma, :],
                    )
            return s_new

        # DMA issue order follows consumption order: the ma=0 W quarter and
        # the narrow x0 tile gate the first matmul group; later quarters and
        # tiles arrive while earlier groups run; j/b are needed only once
        # settling starts.
        QW = KM * 128  # columns per w quarter (one ma)

        def load_w(lo, hi):
            nc.sync.dma_start(out=w_t[:, lo:hi], in_=wd[:, lo:hi])

        load_w(0, QW)
        xs = {0: load_x(0, 2)}
        load_w(QW, 2 * QW)
        load_w(2 * QW, 3 * QW)
        load_w(3 * QW, 4 * QW)
        xs[1] = load_x(1, 2)
        nc.sync.dma_start(out=j_t, in_=jd.ap())
        xs[2] = load_x(2, 2)
        nc.sync.dma_start(out=b_t, in_=bd.ap())

        for gi, group in enumerate(GROUPS):
            dts, sts = {}, {}
            for ti in group:
                dts[ti], sts[ti] = drive_and_sig1(ti, xs.pop(ti))
            if gi + 1 < len(GROUPS):
                for ti in GROUPS[gi + 1]:
                    xs[ti] = load_x(ti, 2)
            for step in range(STEPS - 1):
                for ti in group:
                    sts[ti] = settle_step(
                        ti, dts[ti], sts[ti], last=step == STEPS - 2
                    )

    nc.finalize()
    return nc


def _get_nc():
    if "nc" not in _CACHE:
        _CACHE["nc"] = _build_nc()
    return _CACHE["nc"]


def _run(in_maps, **kwargs):
    import sys

    for p in ("/opt/trn_rl_repo",):
        if p not in sys.path:
            sys.path.append(p)
    from concourse.bass_utils import run_bass_kernel_spmd

    return run_bass_kernel_spmd(_get_nc(), in_maps, list(range(NCORES)), **kwargs)


def _to_sbuf_image(mat, kchunks):
    """[kchunks*128, F] -> [128, kchunks*F] (partition-major SBUF image)."""
    k, f = kchunks, mat.shape[1]
    return (
        mat.reshape(k, 128, f).transpose(1, 0, 2).reshape(128, k * f)
    )


def _make_in_maps(x, W_in, b_in, J):
    import ml_dtypes

    bf = ml_dtypes.bfloat16
    x = np.asarray(x, dtype=np.float32)
    # w image is ma-major: [128, (ma, k, col)] with block (ma, k) holding
    # W_in^T[k*128 + p, ma*128 + col].
    wT = np.asarray(W_in, dtype=np.float32).T.reshape(KM, 128, KA, 128)
    wd = np.ascontiguousarray(
        wT.transpose(1, 2, 0, 3).reshape(128, KA * KM * 128)
    ).astype(bf)
    jd = np.ascontiguousarray(
        _to_sbuf_image(np.asarray(J, dtype=np.float32).T / TAU, KA)
    ).astype(bf)
    bd = np.ascontiguousarray(
        np.asarray(b_in, dtype=np.float32).reshape(KA, 128).T
    )
    in_maps = []
    for c in range(NCORES):
        xT = x[c * BSH : (c + 1) * BSH, :].T  # [M, BSH]
        xpk = xT.reshape(KM, 128, BSH).transpose(1, 0, 2)  # [128, KM, BSH]
        blocks = [
            xpk[:, :, off : off + w].reshape(128, KM * w) for off, w in TILES
        ]
        xdc = np.ascontiguousarray(np.concatenate(blocks, axis=1)).astype(bf)
        in_maps.append({"xd": xdc, "wd": wd, "jd": jd, "bd": bd})
    return in_maps


def _assemble(results):
    # sd [128, KA*BSH] per core -> sigma^T [A, BSH] -> concat -> [B, A]
    parts = []
    for c in range(NCORES):
        sdc = results[c]["sd"].reshape(128, KA, BSH)
        parts.append(sdc.transpose(1, 0, 2).reshape(A, BSH))
    sigT = np.concatenate(parts, axis=1)
    return np.ascontiguousarray(sigT.T).astype(np.float32)  # [B, A]


def kernel(x, W_in, b_in, J):
    res = _run(_make_in_maps(x, W_in, b_in, J))
    return _assemble(res.results)


# revision 34
# speedup vs baseline: 1.0064x; 1.0032x over previous
"""AttractorDynamics Trainium2 kernel.

Reference computation (B=16384, M=1024, A=512, tau=0.1, 10 settling steps):
    drive = x @ W_in.T + b_in                     # [B, A]
    sigma = 0
    repeat 10: sigma = tanh(drive + (sigma @ J.T) / tau)

Strategy: data-parallel over batch across 8 NeuronCores (2048 rows each).
Everything is kept in a transposed [feature, batch] layout so every matmul
has the contraction dim on partitions and no on-device transposes are
needed:
    drive^T[a, b] = sum_m W_in^T[m, a] * x^T[m, b]
    (sigma @ Js^T)^T[a', b] = sum_a Js^T[a, a'] * sigma^T[a, b]
with Js = J / tau folded on the host. Matmul operands are bf16 (full-rate
PE, fast weight loads) with fp32 PSUM accumulation; drive is kept fp32 and
the final settling step is computed and stored in fp32.

All DRAM tensors are pre-arranged on the host into SBUF-image layout
([128 partitions, flat free dim], per-tile contiguous) so every DMA moves
128 rows of multi-KB contiguous data at full queue bandwidth.

Per core the 2048 batch columns are cut into tiles [256, 512, 512, 512,
256] processed as two interleaved groups (3 then 2 tiles) so the tensor
engine always has another tile's matmuls to run while a tile's add+tanh
tail executes. The narrow first tile gets the first matmul started sooner
after the DMA preamble; the narrow last tile shortens the final drain.
Output chunks stream out on alternating DMA queues as soon as their final
tanh lands.
"""

import numpy as np

B, M, A = 16384, 1024, 512
TAU = 0.1
STEPS = 10
NCORES = 8
BSH = B // NCORES  # 2048 batch rows per core
KM = M // 128  # 8 contraction chunks for the input projection
KA = A // 128  # 4 contraction chunks for the recurrence

# (column offset, width) per batch tile; groups are processed sequentially
# with round-robin settling inside each group.
TILES = [(0, 256), (256, 512), (768, 512), (1280, 512), (1792, 256)]
GROUPS = [(0, 1, 2), (3, 4)]
# start of each tile's block in the tile-contiguous x layout (in elements
# of the free dim, per partition)
XPOS = [off * KM for off, _ in TILES]

_CACHE = {}


def _build_nc():
    import sys

    for p in ("/opt/trn_rl_repo",):
        if p not in sys.path:
            sys.path.append(p)
    import concourse.tile as tile
    from concourse import bacc, mybir

    f32 = mybir.dt.float32
    bf16 = mybir.dt.bfloat16
    Tanh = mybir.ActivationFunctionType.Tanh

    nc = bacc.Bacc(None)
    # All tensors in [128, flat] SBUF-image layout (see _make_in_maps).
    xd = nc.dram_tensor("xd", [128, KM * BSH], bf16, kind="ExternalInput")
    wd = nc.dram_tensor("wd", [128, KM * A], bf16, kind="ExternalInput")
    jd = nc.dram_tensor("jd", [128, KA * A], bf16, kind="ExternalInput")
    bd = nc.dram_tensor("bd", [128, KA], f32, kind="ExternalInput")
    sd = nc.dram_tensor("sd", [128, KA * BSH], f32, kind="ExternalOutput")

    with (
        tile.TileContext(nc) as tc,
        tc.tile_pool(name="consts", bufs=1) as consts,
        tc.tile_pool(name="xp", bufs=5) as xp,
        tc.tile_pool(name="dp", bufs=5) as dp,
        tc.tile_pool(name="sp", bufs=8) as sp,
        tc.tile_pool(name="fp", bufs=3) as fp,
        tc.tile_pool(name="psp", bufs=8, space="PSUM") as psp,
    ):
        # w is stored ma-major: block (ma, k) of [128, 128] lives at
        # [:, (ma*KM + k)*128]; the first drive group only needs the ma=0
        # quarter, so quarter-wise loads unblock the first matmuls early.
        w_t = consts.tile([128, KA * KM * 128], bf16)
        j_t = consts.tile([128, KA * A], bf16)
        b_t = consts.tile([128, KA], f32)

        def load_x(ti, nsplit, engines=None):
            _, w = TILES[ti]
            x_t = xp.tile([128, KM * w], bf16, tag="x_t")
            step = KM * w // nsplit
            for h in range(nsplit):
                eng = engines[h] if engines else nc.sync
                eng.dma_start(
                    out=x_t[:, h * step : (h + 1) * step],
                    in_=xd[:, XPOS[ti] + h * step : XPOS[ti] + (h + 1) * step],
                )
            return x_t

        def drive_and_sig1(ti, x_t):
            # drive = x @ W_in.T + b (kept for all steps); sigma_1 = tanh(drive)
            _, w = TILES[ti]
            d_t = dp.tile([128, KA, w], f32, tag="d_t")
            s_t = sp.tile([128, KA, w], bf16, tag="s_t")
            for ma in range(KA):
                ps = psp.tile([128, w], f32, tag="ps")
                for k in range(KM):
                    blk = (ma * KM + k) * 128
                    nc.tensor.matmul(
                        ps,
                        lhsT=w_t[:, blk : blk + 128],
                        rhs=x_t[:, k * w : (k + 1) * w],
                        start=(k == 0),
                        stop=(k == KM - 1),
                    )
                nc.vector.tensor_copy(d_t[:, ma, :], ps)
                nc.scalar.activation(
                    out=s_t[:, ma, :],
                    in_=ps,
                    func=Tanh,
                    bias=b_t[:, ma : ma + 1],
                    scale=1.0,
                )
            return d_t, s_t

        def settle_step(ti, d_t, s_t, last):
            off, w = TILES[ti]
            if last:
                s_new = fp.tile([128, KA, w], f32, tag="f_t")
            else:
                s_new = sp.tile([128, KA, w], bf16, tag="s_t")
            for ma in range(KA):
                ps = psp.tile([128, w], f32, tag="ps")
                for ka in range(KA):
                    nc.tensor.matmul(
                        ps,
                        lhsT=j_t[:, ka * A + ma * 128 : ka * A + (ma + 1) * 128],
                        rhs=s_t[:, ka, :],
                        start=(ka == 0),
                        stop=(ka == KA - 1),
                    )
                nc.vector.tensor_add(s_new[:, ma, :], ps, d_t[:, ma, :])
                nc.scalar.activation(
                    out=s_new[:, ma, :],
                    in_=s_new[:, ma, :],
                    func=Tanh,
                    bias=b_t[:, ma : ma + 1],
                    scale=1.0,
                )
                if last:
                    # Stream each finished chunk out immediately via the
                    # hardware-DGE path so the final flush is fast.
                    nc.sync.dma_start(
                        out=sd[:, ma * BSH + off : ma * BSH + off + w],
                        in_=s_new[:, ma, :],
                    )
            return s_new

        # DMA issue order follows consumption order: the ma=0 W quarter and
        # the narrow x0 tile gate the first matmul group; later quarters and
        # tiles arrive while earlier groups run; j/b are needed only once
        # settling starts.
        QW = KM * 128  # columns per w quarter (one ma)

        def load_w(lo, hi):
            nc.sync.dma_start(out=w_t[:, lo:hi], in_=wd[:, lo:hi])

        load_w(0, QW)
        xs = {0: load_x(0, 2)}
        load_w(QW, 2 * QW)
        load_w(2 * QW, 3 * QW)
        load_w(3 * QW, 4 * QW)
        xs[1] = load_x(1, 2)
        nc.sync.dma_start(out=j_t, in_=jd.ap())
        xs[2] = load_x(2, 2)
        nc.sync.dma_start(out=b_t, in_=bd.ap())

        for gi, group in enumerate(GROUPS):
            dts, sts = {}, {}
            for ti in group:
                dts[ti], sts[ti] = drive_and_sig1(ti, xs.pop(ti))
            if gi + 1 < len(GROUPS):
                for ti in GROUPS[gi + 1]:
                    xs[ti] = load_x(ti, 2)
            for step in range(STEPS - 1):
                for ti in group:
                    sts[ti] = settle_step(
                        ti, dts[ti], sts[ti], last=step == STEPS - 2
                    )

    nc.finalize()
    return nc


def _get_nc():
    if "nc" not in _CACHE:
        _CACHE["nc"] = _build_nc()
    return _CACHE["nc"]


def _run(in_maps, **kwargs):
    import sys

    for p in ("/opt/trn_rl_repo",):
        if p not in sys.path:
            sys.path.append(p)
    from concourse.bass_utils import run_bass_kernel_spmd

    return run_bass_kernel_spmd(_get_nc(), in_maps, list(range(NCORES)), **kwargs)


def _to_sbuf_image(mat, kchunks):
    """[kchunks*128, F] -> [128, kchunks*F] (partition-major SBUF image)."""
    k, f = kchunks, mat.shape[1]
    return (
        mat.reshape(k, 128, f).transpose(1, 0, 2).reshape(128, k * f)
    )


def _make_in_maps(x, W_in, b_in, J):
    import ml_dtypes

    bf = ml_dtypes.bfloat16
    x = np.asarray(x, dtype=np.float32)
    # w image is ma-major: [128, (ma, k, col)] with block (ma, k) holding
    # W_in^T[k*128 + p, ma*128 + col].
    wT = np.asarray(W_in, dtype=np.float32).T.reshape(KM, 128, KA, 128)
    wd = np.ascontiguousarray(
        wT.transpose(1, 2, 0, 3).reshape(128, KA * KM * 128)
    ).astype(bf)
    jd = np.ascontiguousarray(
        _to_sbuf_image(np.asarray(J, dtype=np.float32).T / TAU, KA)
    ).astype(bf)
    bd = np.ascontiguousarray(
        np.asarray(b_in, dtype=np.float32).reshape(KA, 128).T
    )
    in_maps = []
    for c in range(NCORES):
        xT = x[c * BSH : (c + 1) * BSH, :].T  # [M, BSH]
        xpk = xT.reshape(KM, 128, BSH).transpose(1, 0, 2)  # [128, KM, BSH]
        blocks = [
            xpk[:, :, off : off + w].reshape(128, KM * w) for off, w in TILES
        ]
        xdc = np.ascontiguousarray(np.concatenate(blocks, axis=1)).astype(bf)
        in_maps.append({"xd": xdc, "wd": wd, "jd": jd, "bd": bd})
    return in_maps


def _assemble(results):
    # sd [128, KA*BSH] per core -> sigma^T [A, BSH] -> concat -> [B, A]
    parts = []
    for c in range(NCORES):
        sdc = results[c]["sd"].reshape(128, KA, BSH)
        parts.append(sdc.transpose(1, 0, 2).reshape(A, BSH))
    sigT = np.concatenate(parts, axis=1)
    return np.ascontiguousarray(sigT.T).astype(np.float32)  # [B, A]


def kernel(x, W_in, b_in, J):
    res = _run(_make_in_maps(x, W_in, b_in, J))
    return _assemble(res.results)
